# revision 14
# baseline (speedup 1.0000x reference)
"""CongestionGCN on 8 Trainium2 NeuronCores.

Graph/data-parallel sharding: nodes split contiguously across 8 cores
(12500 each, padded to 12544 = 98*128). Edges partitioned by dst node.

- Layer 1's message aggregation is linear in the raw inputs, so the
  host folds it away: both the Wself and Wneigh contributions collapse
  to two 13-row GEMMs against host-shipped [X^T;1] and [(S_w X)^T;mask].
  No gather, no AllGather, no embedding GEMM for layer 1.
- Layers 2/3 message passing: indirect-DMA gathers of src rows from a
  node-major table (4 windows = src quarters across all cores, int16
  addressable). Gathers are merged per (2-group wave, window) and
  spread across the 4 SWDGE queues so the Q7 pairs generate
  descriptors in parallel. Host-precomputed one-hot matrices (layer
  invariant, inv_deg folded in, bf16) are streamed from DRAM; a
  matmul against them performs scatter-add + mean scaling,
  accumulating msg^T in PSUM.
- Each layer's output is transposed back to node-major and AllGathered
  in 4 quarter chunks interleaved with phase 2, so window-q gathers of
  the next layer start as soon as chunk q lands.
- BN+ReLU is fused into one scalar-engine activation (out =
  relu(scale*x + bias)); batch stats cross-core via AllReduce.
  conv_b is dropped: BN subtracts the batch mean, so a pre-BN
  per-feature bias cancels exactly.
"""

import numpy as np

N = 100000
E = 600000
IN = 12
H = 128
OUT = 2
L = 3
EPS = 1e-5

NCORES = 8
P = 128
NPC = N // NCORES            # 12500 real nodes per core
NT = (NPC + P - 1) // P      # 98 dst tiles per core
NPC_PAD = NT * P             # 12544
GW = 512                     # free-dim group width

NW = 4                       # src windows = shard quarters
QB = [0, 3200, 6400, 9472, 12544]      # quarter boundaries (128-aligned)
QS = [3200, 3200, 3072, 3072]
WR = [8 * q for q in QS]               # window table rows (<= int16 range)
QT = [25, 25, 24, 24]                  # tiles per quarter
NG = (NT + 3) // 4           # 25 groups of up to 4 dst tiles
WAVES = [(2 * i, 2) for i in range(12)] + [(24, 1)]
NVW = len(WAVES)             # 13 gather waves (2 groups each, last 1)

_cache = {}


def _host_prep(features, edge_index, emb_W, emb_b, Wself, Wneigh, **kw):
    import ml_dtypes

    src = np.asarray(edge_index[0], dtype=np.int64)
    dst = np.asarray(edge_index[1], dtype=np.int64)
    X = np.asarray(features, dtype=np.float32)

    deg = np.bincount(dst, minlength=N).astype(np.float32)
    inv_deg = (1.0 / np.maximum(deg, 1.0)).astype(np.float32)

    # ---- layer-1 folding: SwX = diag(inv_deg) * segment_sum(X[src], dst)
    SwX = np.empty((N, IN), np.float32)
    for j in range(IN):
        SwX[:, j] = np.bincount(dst, weights=X[src, j], minlength=N)
    SwX *= inv_deg[:, None]
    mask = (deg > 0).astype(np.float32)

    emb_W = np.asarray(emb_W, np.float32)
    emb_b = np.asarray(emb_b, np.float32)
    Ws0 = np.asarray(Wself[0], np.float32)
    Wn0 = np.asarray(Wneigh[0], np.float32)
    lhs1 = np.vstack([emb_W @ Ws0, emb_b[None, :] @ Ws0])   # [13, H]
    lhs2 = np.vstack([emb_W @ Wn0, emb_b[None, :] @ Wn0])   # [13, H]

    featT = np.zeros((NCORES, IN + 1, NPC_PAD), np.float32)
    hXT = np.zeros((NCORES, IN + 1, NPC_PAD), np.float32)
    for c in range(NCORES):
        sl = slice(c * NPC, (c + 1) * NPC)
        featT[c, :IN, :NPC] = X[sl].T
        featT[c, IN, :NPC] = 1.0
        hXT[c, :IN, :NPC] = SwX[sl].T
        hXT[c, IN, :NPC] = mask[sl]

    # ---- edge partitioning for layers 2/3 gathers
    core = dst // NPC
    ltile = (dst % NPC) // P                 # 0..97
    group = ltile // 4                       # 0..24
    tloc = ltile % 4
    dst_rel = (dst % NPC) % P
    src_c = src // NPC
    src_r = src % NPC
    win = np.searchsorted(QB, src_r, side='right') - 1
    qs = np.asarray(QS, np.int64)
    qb = np.asarray(QB[:4], np.int64)
    idx_rel = (src_c * qs[win] + (src_r - qb[win])).astype(np.int16)

    v = group // 2                           # wave id
    gt = (group % 2) * 4 + tloc              # tile within wave, 0..7

    cell = ((core * NVW + v) * NW + win) * 8 + gt
    ncell = NCORES * NVW * NW * 8
    order = np.lexsort((src, cell))
    cell_s = cell[order]
    counts = np.bincount(cell_s, minlength=ncell)
    C_tw = int((counts.max() + P - 1) // P)
    CAPS = C_tw * P
    starts = np.zeros(ncell, dtype=np.int64)
    starts[1:] = np.cumsum(counts)[:-1]
    pos = np.arange(E, dtype=np.int64) - starts[cell_s]

    idx_s = idx_rel[order]
    w_s = inv_deg[dst[order]]
    core_s = core[order]
    v_s = v[order]
    win_s = win[order]
    gt_s = gt[order]

    slot = gt_s * CAPS + pos
    # idxs per (wave, window): 8*CAPS for full waves, 2*CAPS for the last
    wc = np.where(v_s == NVW - 1, 2 * CAPS // 16, 8 * CAPS // 16)
    WCMAX = 8 * CAPS // 16
    idx16 = np.zeros((NCORES, NVW, 16, 4 * WCMAX), dtype=np.int16)
    idx16[core_s, v_s, slot % 16, win_s * wc + slot // 16] = idx_s

    # one-hot scatter matrices (layer-invariant, inv_deg folded, bf16):
    # chunk id m = (gt*NW + win)*C_tw + cc
    cc = pos // P
    m = (gt_s * NW + win_s) * C_tw + cc
    CPW = 8 * NW * C_tw                      # chunks per wave (max)
    oh = np.zeros((NCORES, NVW, P, CPW * P), dtype=ml_dtypes.bfloat16)
    oh[core_s, v_s, pos % P, m * P + dst_rel[order]] = w_s

    idx16_full = np.broadcast_to(
        idx16[:, :, None, :, :], (NCORES, NVW, 8, 16, 4 * WCMAX)
    ).reshape(NCORES, NVW, P, 4 * WCMAX)

    return (np.ascontiguousarray(idx16_full), oh, featT, hXT,
            lhs1, lhs2, C_tw)


def _build_program(C):  # C == C_tw
    import concourse.bacc as bacc
    import concourse.tile as tile
    from concourse import bass, mybir
    from concourse.masks import make_identity

    f32 = mybir.dt.float32
    i16 = mybir.dt.int16
    gd = mybir.dt.bfloat16

    nc = bacc.Bacc("TRN2", target_bir_lowering=False, num_swdge_queues=4)

    C_tw = C
    CAPS = C_tw * P
    WCMAX = 8 * CAPS // 16
    CPW = 8 * NW * C_tw

    featT_p = nc.declare_dram_parameter("featT", [IN + 1, NPC_PAD], f32, isOutput=False)
    hXT_p = nc.declare_dram_parameter("hXT", [IN + 1, NPC_PAD], f32, isOutput=False)
    lhs1_p = nc.declare_dram_parameter("lhs1", [IN + 1, H], f32, isOutput=False)
    lhs2_p = nc.declare_dram_parameter("lhs2", [IN + 1, H], f32, isOutput=False)
    idx_p = nc.declare_dram_parameter("idx", [NVW, P, 4 * WCMAX], i16, isOutput=False)
    oh_p = nc.declare_dram_parameter("oh", [NVW, P, CPW * P], gd, isOutput=False)
    Wself_p = nc.declare_dram_parameter("Wself", [L, H, H], f32, isOutput=False)
    Wneigh_p = nc.declare_dram_parameter("Wneigh", [L, H, H], f32, isOutput=False)
    bng_p = nc.declare_dram_parameter("bng", [L, H, 1], f32, isOutput=False)
    bnb_p = nc.declare_dram_parameter("bnb", [L, H, 1], f32, isOutput=False)
    W1_p = nc.declare_dram_parameter("W1", [H, H // 2], f32, isOutput=False)
    b1_p = nc.declare_dram_parameter("b1", [H // 2, 1], f32, isOutput=False)
    W2_p = nc.declare_dram_parameter("W2", [H // 2, OUT], f32, isOutput=False)
    b2_p = nc.declare_dram_parameter("b2", [OUT, 1], f32, isOutput=False)
    out_p = nc.declare_dram_parameter("out", [OUT, NPC_PAD], f32, isOutput=True)

    groups = [(s, min(GW, NPC_PAD - s)) for s in range(0, NPC_PAD, GW)]
    rg = [list(range(NCORES))]
    # quarter q's transposes can run once phase 2 has covered its columns
    qtrig = [min((QB[q + 1] + GW - 1) // GW, len(groups)) - 1 for q in range(NW)]

    with tile.TileContext(nc) as tc:
        with (
            tc.tile_pool(name="persist", bufs=1) as pp,
            tc.tile_pool(name="pdram", bufs=1, space="DRAM") as pd,
            tc.tile_pool(name="gpool", bufs=8) as gpool,
            tc.tile_pool(name="ohpool", bufs=2) as ohpool,
            tc.tile_pool(name="mpool", bufs=3) as mpool,
            tc.tile_pool(name="grp", bufs=2) as grp,
            tc.tile_pool(name="small", bufs=8) as small,
            tc.tile_pool(name="trp", bufs=4) as trp,
            tc.tile_pool(name="mps", bufs=4, space="PSUM") as mps,
            tc.tile_pool(name="xps", bufs=2, space="PSUM") as xps,
            tc.tile_pool(name="tps", bufs=2, space="PSUM") as tps,
        ):
            # --- persistent SBUF state ---
            bufA = pp.tile([P, NPC_PAD], f32, tag="bufA", name="bufA")
            bufB = pp.tile([P, NPC_PAD], gd, tag="bufB", name="bufB")
            ident = pp.tile([P, P], f32, tag="ident", name="ident")
            make_identity(nc, ident[:])
            eps_t = pp.tile([P, 1], f32, tag="eps_t", name="eps_t")
            nc.gpsimd.memset(eps_t[:], EPS)

            lhs1_t = pp.tile([IN + 1, H], f32, tag="lhs1_t", name="lhs1_t")
            nc.sync.dma_start(lhs1_t[:], lhs1_p[:])
            lhs2_t = pp.tile([IN + 1, H], f32, tag="lhs2_t", name="lhs2_t")
            nc.sync.dma_start(lhs2_t[:], lhs2_p[:])
            W1_t = pp.tile([H, H // 2], f32, tag="W1_t", name="W1_t")
            nc.sync.dma_start(W1_t[:], W1_p[:])
            b1_t = pp.tile([H // 2, 1], f32, tag="b1_t", name="b1_t")
            nc.sync.dma_start(b1_t[:], b1_p[:])
            W2_t = pp.tile([H // 2, OUT], f32, tag="W2_t", name="W2_t")
            nc.sync.dma_start(W2_t[:], W2_p[:])
            b2_t = pp.tile([OUT, 1], f32, tag="b2_t", name="b2_t")
            nc.sync.dma_start(b2_t[:], b2_p[:])

            # --- internal DRAM: per-quarter AG inputs + window tables ---
            ag_q = [
                [pd.tile([QS[q], H], gd, tag=f"ag{l}_{q}", name=f"ag{l}_{q}")
                 for q in range(NW)]
                for l in range(2)
            ]
            tab = [
                [pd.tile([WR[q], H], gd, addr_space="Shared",
                         tag=f"tab{l}_{q}", name=f"tab{l}_{q}")
                 for q in range(NW)]
                for l in range(2)
            ]
            ar_in = [
                pd.tile([P, 2], f32, tag=f"ar_in{l}", name=f"ar_in{l}")
                for l in range(L)
            ]
            ar_out = [
                pd.tile([P, 2], f32, addr_space="Shared",
                        tag=f"ar_out{l}", name=f"ar_out{l}")
                for l in range(L)
            ]

            def transpose_quarter(l, q):
                kt = sum(QT[:q])
                nt_q = QT[q]
                done = 0
                while done < nt_q:
                    nstage = min(4, nt_q - done)
                    stage = trp.tile([P, GW], gd, tag="tr")
                    for k in range(nstage):
                        t = kt + done + k
                        ps = tps.tile([P, P], f32, tag="tps")
                        nc.tensor.transpose(
                            out=ps[:], in_=bufA[:, t * P:(t + 1) * P],
                            identity=ident[:],
                        )
                        nc.vector.tensor_copy(
                            out=stage[:, k * P:(k + 1) * P], in_=ps[:])
                    so = done * P
                    w = nstage * P
                    nc.scalar.dma_start(
                        ag_q[l][q][so:so + w, :].rearrange(
                            "(b p) f -> p b f", p=P),
                        stage[:, :w].rearrange("p (b f) -> p b f", f=P),
                    )
                    done += nstage
                nc.gpsimd.collective_compute(
                    "AllGather",
                    mybir.AluOpType.bypass,
                    ins=[ag_q[l][q].opt()],
                    outs=[tab[l][q].opt()],
                    replica_groups=rg,
                )

            def head_group(s, w):
                ps1 = xps.tile([H // 2, GW], f32, tag="xps")
                nc.tensor.matmul(out=ps1[:, :w], lhsT=W1_t[:],
                                 rhs=bufA[:, s:s + w], start=True, stop=True)
                z1 = grp.tile([H // 2, GW], f32, tag="z1")
                nc.scalar.activation(
                    z1[:, :w], ps1[:, :w],
                    mybir.ActivationFunctionType.Relu, bias=b1_t[:],
                )
                ps2 = tps.tile([OUT, GW], f32, tag="tps")
                nc.tensor.matmul(out=ps2[:, :w], lhsT=W2_t[:],
                                 rhs=z1[:, :w], start=True, stop=True)
                o = trp.tile([OUT, GW], f32, tag="tro")
                nc.vector.tensor_scalar_add(o[:, :w], ps2[:, :w], b2_t[:])
                nc.sync.dma_start(out_p[:, s:s + w], o[:, :w])

            # ---------------- conv layers ----------------
            for l in range(L):
                if l > 0:
                    Wself_t = small.tile([H, H], f32, tag="ws", bufs=2)
                    nc.sync.dma_start(Wself_t[:], Wself_p[l])
                    Wneigh_t = small.tile([H, H], f32, tag="wn", bufs=2)
                    nc.sync.dma_start(Wneigh_t[:], Wneigh_p[l])
                bng_t = small.tile([H, 1], f32, tag="bng", bufs=2)
                nc.sync.dma_start(bng_t[:], bng_p[l])
                bnb_t = small.tile([H, 1], f32, tag="bnb", bufs=2)
                nc.sync.dma_start(bnb_t[:], bnb_p[l])

                s1p = small.tile([P, 32], f32, tag="s1p", bufs=2)
                s2p = small.tile([P, 32], f32, tag="s2p", bufs=2)

                def stats_and_stash(ps, gi, s, w):
                    wr = w if s + w <= NPC else max(0, NPC - s)
                    if wr > 0:
                        nc.vector.reduce_sum(
                            s1p[:, gi:gi + 1], ps[:, :wr],
                            axis=mybir.AxisListType.X,
                        )
                        sq = grp.tile([P, GW], f32, tag="sq")
                        nc.scalar.activation(
                            sq[:, :wr], ps[:, :wr],
                            mybir.ActivationFunctionType.Square,
                            accum_out=s2p[:, gi:gi + 1],
                        )
                    nc.vector.tensor_copy(out=bufB[:, s:s + w], in_=ps[:, :w])

                # ---- phase 1: messages + GEMM + stats ----
                if l == 0:
                    for gi, (s, w) in enumerate(groups):
                        fa = mpool.tile([IN + 1, GW], f32, tag="fa")
                        nc.sync.dma_start(fa[:, :w], featT_p[:, s:s + w])
                        fx = mpool.tile([IN + 1, GW], f32, tag="fx")
                        nc.sync.dma_start(fx[:, :w], hXT_p[:, s:s + w])
                        ps = xps.tile([P, GW], f32, tag="xps")
                        nc.tensor.matmul(out=ps[:, :w], lhsT=lhs1_t[:],
                                         rhs=fa[:, :w], start=True, stop=False)
                        nc.tensor.matmul(out=ps[:, :w], lhsT=lhs2_t[:],
                                         rhs=fx[:, :w], start=False, stop=True)
                        stats_and_stash(ps, gi, s, w)
                else:
                    for v, (g0, ngr) in enumerate(WAVES):
                        ngtiles = 8 if ngr == 2 else 2
                        wcols = ngtiles * CAPS // 16
                        nidx = ngtiles * CAPS
                        it = mpool.tile([P, 4 * WCMAX], i16, tag="idx")
                        nc.sync.dma_start(it[:, :4 * wcols],
                                          idx_p[v][:, :4 * wcols])
                        oh_t = ohpool.tile([P, CPW * P], gd, tag="oh")
                        ncols = ngtiles * NW * C_tw * P
                        oh_eng = nc.sync if v % 2 == 0 else nc.scalar
                        oh_eng.dma_start(oh_t[:, :ncols], oh_p[v][:, :ncols])
                        # one gather per (window, group): 4*CAPS idxs each
                        # stays under the 128-per-engine SWDGE ring window
                        gcols = 4 * CAPS // 16
                        Gs = {}
                        for wi in range(NW):
                            for gw in range(ngr):
                                gt_n = min(4, ngtiles - gw * 4)
                                ni = gt_n * CAPS
                                gw_t = gpool.tile([P, 4 * C_tw * H], gd,
                                                  tag="g")
                                cb = wi * wcols + gw * gcols
                                nc.gpsimd.dma_gather(
                                    out_ap=gw_t[:, :ni // P * H].rearrange(
                                        "p (c e) -> p c e", e=H),
                                    in_ap=tab[l - 1][wi][:],
                                    idxs_ap=it[:, cb:cb + ni // 16],
                                    num_idxs=ni,
                                    num_idxs_reg=ni,
                                    elem_size=H,
                                    queue_num=wi,
                                )
                                Gs[(wi, gw)] = gw_t
                        for gw in range(ngr):
                            gi = g0 + gw
                            s, w = groups[gi]
                            ntg = (w + P - 1) // P
                            msg_ps = mps.tile([P, GW], f32, tag="mps")
                            nmm = NW * C_tw
                            for t in range(ntg):
                                i_mm = 0
                                for wi in range(NW):
                                    for cc2 in range(C_tw):
                                        gt = gw * 4 + t
                                        ci = t * C_tw + cc2
                                        m = (gt * NW + wi) * C_tw + cc2
                                        nc.tensor.matmul(
                                            out=msg_ps[:, t * P:(t + 1) * P],
                                            lhsT=Gs[(wi, gw)][:,
                                                             ci * H:(ci + 1) * H],
                                            rhs=oh_t[:, m * P:(m + 1) * P],
                                            start=(i_mm == 0),
                                            stop=(i_mm == nmm - 1),
                                        )
                                        i_mm += 1
                            msg_g = grp.tile([P, GW], f32, tag="msg")
                            nc.scalar.copy(out=msg_g[:, :w], in_=msg_ps[:, :w])
                            ps = xps.tile([P, GW], f32, tag="xps")
                            nc.tensor.matmul(out=ps[:, :w], lhsT=Wself_t[:],
                                             rhs=bufA[:, s:s + w],
                                             start=True, stop=False)
                            nc.tensor.matmul(out=ps[:, :w], lhsT=Wneigh_t[:],
                                             rhs=msg_g[:, :w],
                                             start=False, stop=True)
                            stats_and_stash(ps, gi, s, w)

                # ---- BN stats all-reduce ----
                st = small.tile([P, 2], f32, tag="st", bufs=2)
                nc.vector.reduce_sum(st[:, 0:1], s1p[:, :len(groups)],
                                     axis=mybir.AxisListType.X)
                nc.vector.reduce_sum(st[:, 1:2], s2p[:, :len(groups)],
                                     axis=mybir.AxisListType.X)
                nc.sync.dma_start(ar_in[l][:], st[:])
                nc.gpsimd.collective_compute(
                    "AllReduce",
                    mybir.AluOpType.add,
                    ins=[ar_in[l].opt()],
                    outs=[ar_out[l].opt()],
                    replica_groups=rg,
                )
                sg = small.tile([P, 2], f32, tag="sg", bufs=2)
                nc.sync.dma_start(sg[:], ar_out[l][:])

                mu = small.tile([P, 1], f32, tag="mu", bufs=2)
                nc.vector.tensor_scalar_mul(mu[:], sg[:, 0:1], 1.0 / N)
                ex2 = small.tile([P, 1], f32, tag="ex2", bufs=2)
                nc.vector.tensor_scalar_mul(ex2[:], sg[:, 1:2], 1.0 / N)
                var = small.tile([P, 1], f32, tag="var", bufs=2)
                nc.vector.tensor_tensor(out=var[:], in0=mu[:], in1=mu[:],
                                        op=mybir.AluOpType.mult)
                nc.vector.tensor_tensor(out=var[:], in0=ex2[:], in1=var[:],
                                        op=mybir.AluOpType.subtract)
                sd = small.tile([P, 1], f32, tag="sd", bufs=2)
                nc.scalar.activation(sd[:], var[:],
                                     mybir.ActivationFunctionType.Sqrt,
                                     bias=eps_t[:])
                rstd = small.tile([P, 1], f32, tag="rstd", bufs=2)
                nc.vector.reciprocal(rstd[:], sd[:])
                a_t = small.tile([P, 1], f32, tag="a_t", bufs=2)
                nc.vector.tensor_tensor(out=a_t[:], in0=bng_t[:], in1=rstd[:],
                                        op=mybir.AluOpType.mult)
                b_t = small.tile([P, 1], f32, tag="b_t", bufs=2)
                nc.vector.tensor_tensor(out=b_t[:], in0=mu[:], in1=a_t[:],
                                        op=mybir.AluOpType.mult)
                nc.vector.tensor_tensor(out=b_t[:], in0=bnb_t[:], in1=b_t[:],
                                        op=mybir.AluOpType.subtract)

                # ---- phase 2: fused BN+ReLU (+ residual), interleaved with
                # per-quarter transposes + chunked AllGather ----
                nextq = 0
                for gi, (s, w) in enumerate(groups):
                    if l == 0:
                        nc.scalar.activation(
                            bufA[:, s:s + w], bufB[:, s:s + w],
                            mybir.ActivationFunctionType.Relu,
                            bias=b_t[:], scale=a_t[:],
                        )
                    else:
                        y2 = grp.tile([P, GW], f32, tag="y2")
                        nc.scalar.activation(
                            y2[:, :w], bufB[:, s:s + w],
                            mybir.ActivationFunctionType.Relu,
                            bias=b_t[:], scale=a_t[:],
                        )
                        nc.vector.tensor_tensor(
                            out=bufA[:, s:s + w], in0=y2[:, :w],
                            in1=bufA[:, s:s + w], op=mybir.AluOpType.add,
                        )
                    if gi == len(groups) - 1:
                        nc.gpsimd.memset(bufA[:, NPC:], 0.0)
                    if l == 2:
                        head_group(s, w)
                    elif nextq < NW and gi == qtrig[nextq]:
                        if gi == len(groups) - 1:
                            pass  # memset already issued above
                        transpose_quarter(l, nextq)
                        nextq += 1

    nc.compile()
    return nc


def kernel(**inputs):
    from concourse.bass_utils import run_bass_kernel_spmd

    idx, oh, featT, hXT, lhs1, lhs2, C = _host_prep(**inputs)

    key = ("prog", C)
    if key not in _cache:
        _cache[key] = _build_program(C)
    nc = _cache[key]

    f32 = np.float32
    Wself = np.ascontiguousarray(np.asarray(inputs["Wself"], f32))
    Wneigh = np.ascontiguousarray(np.asarray(inputs["Wneigh"], f32))
    bng = np.asarray(inputs["bn_gamma"], f32).reshape(L, H, 1)
    bnb = np.asarray(inputs["bn_beta"], f32).reshape(L, H, 1)
    W1 = np.ascontiguousarray(np.asarray(inputs["W1"], f32))
    b1 = np.asarray(inputs["b1"], f32).reshape(H // 2, 1)
    W2 = np.ascontiguousarray(np.asarray(inputs["W2"], f32))
    b2 = np.asarray(inputs["b2"], f32).reshape(OUT, 1)

    in_maps = []
    for c in range(NCORES):
        in_maps.append({
            "featT": np.ascontiguousarray(featT[c]),
            "hXT": np.ascontiguousarray(hXT[c]),
            "lhs1": np.ascontiguousarray(lhs1),
            "lhs2": np.ascontiguousarray(lhs2),
            "idx": np.ascontiguousarray(idx[c]),
            "oh": np.ascontiguousarray(oh[c]),
            "Wself": Wself, "Wneigh": Wneigh,
            "bng": bng, "bnb": bnb,
            "W1": W1, "b1": b1, "W2": W2, "b2": b2,
        })

    global _last_in_maps
    _last_in_maps = in_maps

    res = run_bass_kernel_spmd(nc, in_maps, list(range(NCORES))).results
    out = np.concatenate(
        [res[c]["out"][:, :NPC].T for c in range(NCORES)], axis=0
    )
    return out.astype(np.float32)


if __name__ == "__main__":
    pass


# revision 15
# speedup vs baseline: 1.1792x; 1.1792x over previous
"""CongestionGCN on 8 Trainium2 NeuronCores.

Graph/data-parallel sharding: nodes split contiguously across 8 cores
(12500 each, padded to 12544 = 98*128). Edges partitioned by dst node.

- Layer 1's message aggregation is linear in the raw inputs, so the
  host folds it away: both the Wself and Wneigh contributions collapse
  to two 13-row GEMMs against host-shipped [X^T;1] and [(S_w X)^T;mask].
  No gather, no AllGather, no embedding GEMM for layer 1.
- Layers 2/3 message passing: indirect-DMA gathers of src rows from a
  node-major table (4 windows = src quarters across all cores, int16
  addressable). Gathers are merged per (2-group wave, window) and
  spread across the 4 SWDGE queues so the Q7 pairs generate
  descriptors in parallel. Host-precomputed one-hot matrices (layer
  invariant, inv_deg folded in, bf16) are streamed from DRAM; a
  matmul against them performs scatter-add + mean scaling,
  accumulating msg^T in PSUM.
- Each layer's output is transposed back to node-major and AllGathered
  in 4 quarter chunks interleaved with phase 2, so window-q gathers of
  the next layer start as soon as chunk q lands.
- BN+ReLU is fused into one scalar-engine activation (out =
  relu(scale*x + bias)); batch stats cross-core via AllReduce.
  conv_b is dropped: BN subtracts the batch mean, so a pre-BN
  per-feature bias cancels exactly.
"""

import numpy as np

N = 100000
E = 600000
IN = 12
H = 128
OUT = 2
L = 3
EPS = 1e-5

NCORES = 8
P = 128
NPC = N // NCORES            # 12500 real nodes per core
NT = (NPC + P - 1) // P      # 98 dst tiles per core
NPC_PAD = NT * P             # 12544
GW = 512                     # free-dim group width

NW = 4                       # src windows = shard quarters
QB = [0, 3200, 6400, 9472, 12544]      # quarter boundaries (128-aligned)
QS = [3200, 3200, 3072, 3072]
WR = [8 * q for q in QS]               # window table rows (<= int16 range)
QT = [25, 25, 24, 24]                  # tiles per quarter
NG = (NT + 3) // 4           # 25 groups of up to 4 dst tiles
WAVES = [(2 * i, 2) for i in range(12)] + [(24, 1)]
NVW = len(WAVES)             # 13 gather waves (2 groups each, last 1)

_cache = {}


def _host_prep(features, edge_index, emb_W, emb_b, Wself, Wneigh, **kw):
    import ml_dtypes

    src = np.asarray(edge_index[0], dtype=np.int64)
    dst = np.asarray(edge_index[1], dtype=np.int64)
    X = np.asarray(features, dtype=np.float32)

    deg = np.bincount(dst, minlength=N).astype(np.float32)
    inv_deg = (1.0 / np.maximum(deg, 1.0)).astype(np.float32)

    # ---- layer-1 folding: SwX = diag(inv_deg) * segment_sum(X[src], dst)
    SwX = np.empty((N, IN), np.float32)
    for j in range(IN):
        SwX[:, j] = np.bincount(dst, weights=X[src, j], minlength=N)
    SwX *= inv_deg[:, None]
    mask = (deg > 0).astype(np.float32)

    emb_W = np.asarray(emb_W, np.float32)
    emb_b = np.asarray(emb_b, np.float32)
    Ws0 = np.asarray(Wself[0], np.float32)
    Wn0 = np.asarray(Wneigh[0], np.float32)
    lhs1 = np.vstack([emb_W @ Ws0, emb_b[None, :] @ Ws0])   # [13, H]
    lhs2 = np.vstack([emb_W @ Wn0, emb_b[None, :] @ Wn0])   # [13, H]

    featT = np.zeros((NCORES, IN + 1, NPC_PAD), np.float32)
    hXT = np.zeros((NCORES, IN + 1, NPC_PAD), np.float32)
    for c in range(NCORES):
        sl = slice(c * NPC, (c + 1) * NPC)
        featT[c, :IN, :NPC] = X[sl].T
        featT[c, IN, :NPC] = 1.0
        hXT[c, :IN, :NPC] = SwX[sl].T
        hXT[c, IN, :NPC] = mask[sl]

    # ---- edge partitioning for layers 2/3 gathers
    core = dst // NPC
    ltile = (dst % NPC) // P                 # 0..97
    group = ltile // 4                       # 0..24
    tloc = ltile % 4
    dst_rel = (dst % NPC) % P
    src_c = src // NPC
    src_r = src % NPC
    win = np.searchsorted(QB, src_r, side='right') - 1
    qs = np.asarray(QS, np.int64)
    qb = np.asarray(QB[:4], np.int64)
    idx_rel = (src_c * qs[win] + (src_r - qb[win])).astype(np.int16)

    v = group // 2                           # wave id
    gt = (group % 2) * 4 + tloc              # tile within wave, 0..7

    cell = ((core * NVW + v) * NW + win) * 8 + gt
    ncell = NCORES * NVW * NW * 8
    order = np.lexsort((src, cell))
    cell_s = cell[order]
    counts = np.bincount(cell_s, minlength=ncell)
    C_tw = int((counts.max() + P - 1) // P)
    CAPS = C_tw * P
    starts = np.zeros(ncell, dtype=np.int64)
    starts[1:] = np.cumsum(counts)[:-1]
    pos = np.arange(E, dtype=np.int64) - starts[cell_s]

    idx_s = idx_rel[order]
    w_s = inv_deg[dst[order]]
    core_s = core[order]
    v_s = v[order]
    win_s = win[order]
    gt_s = gt[order]

    slot = gt_s * CAPS + pos
    # idxs per (wave, window): 8*CAPS for full waves, 2*CAPS for the last
    wc = np.where(v_s == NVW - 1, 2 * CAPS // 16, 8 * CAPS // 16)
    WCMAX = 8 * CAPS // 16
    idx16 = np.zeros((NCORES, NVW, 16, 4 * WCMAX), dtype=np.int16)
    idx16[core_s, v_s, slot % 16, win_s * wc + slot // 16] = idx_s

    # one-hot scatter matrices (layer-invariant, inv_deg folded, bf16):
    # chunk id m = (gt*NW + win)*C_tw + cc
    cc = pos // P
    m = (gt_s * NW + win_s) * C_tw + cc
    CPW = 8 * NW * C_tw                      # chunks per wave (max)
    oh = np.zeros((NCORES, NVW, P, CPW * P), dtype=ml_dtypes.bfloat16)
    oh[core_s, v_s, pos % P, m * P + dst_rel[order]] = w_s

    idx16_full = np.broadcast_to(
        idx16[:, :, None, :, :], (NCORES, NVW, 8, 16, 4 * WCMAX)
    ).reshape(NCORES, NVW, P, 4 * WCMAX)

    return (np.ascontiguousarray(idx16_full), oh, featT, hXT,
            lhs1, lhs2, C_tw)


def _build_program(C):  # C == C_tw
    import concourse.bacc as bacc
    import concourse.tile as tile
    from concourse import bass, mybir
    from concourse.masks import make_identity

    f32 = mybir.dt.float32
    i16 = mybir.dt.int16
    gd = mybir.dt.bfloat16

    nc = bacc.Bacc("TRN2", target_bir_lowering=False, num_swdge_queues=4)

    C_tw = C
    CAPS = C_tw * P
    WCMAX = 8 * CAPS // 16
    CPW = 8 * NW * C_tw

    featT_p = nc.declare_dram_parameter("featT", [IN + 1, NPC_PAD], f32, isOutput=False)
    hXT_p = nc.declare_dram_parameter("hXT", [IN + 1, NPC_PAD], f32, isOutput=False)
    lhs1_p = nc.declare_dram_parameter("lhs1", [IN + 1, H], f32, isOutput=False)
    lhs2_p = nc.declare_dram_parameter("lhs2", [IN + 1, H], f32, isOutput=False)
    idx_p = nc.declare_dram_parameter("idx", [NVW, P, 4 * WCMAX], i16, isOutput=False)
    oh_p = nc.declare_dram_parameter("oh", [NVW, P, CPW * P], gd, isOutput=False)
    Wself_p = nc.declare_dram_parameter("Wself", [L, H, H], f32, isOutput=False)
    Wneigh_p = nc.declare_dram_parameter("Wneigh", [L, H, H], f32, isOutput=False)
    bng_p = nc.declare_dram_parameter("bng", [L, H, 1], f32, isOutput=False)
    bnb_p = nc.declare_dram_parameter("bnb", [L, H, 1], f32, isOutput=False)
    W1_p = nc.declare_dram_parameter("W1", [H, H // 2], f32, isOutput=False)
    b1_p = nc.declare_dram_parameter("b1", [H // 2, 1], f32, isOutput=False)
    W2_p = nc.declare_dram_parameter("W2", [H // 2, OUT], f32, isOutput=False)
    b2_p = nc.declare_dram_parameter("b2", [OUT, 1], f32, isOutput=False)
    out_p = nc.declare_dram_parameter("out", [OUT, NPC_PAD], f32, isOutput=True)

    groups = [(s, min(GW, NPC_PAD - s)) for s in range(0, NPC_PAD, GW)]
    rg = [list(range(NCORES))]
    # quarter q's transposes can run once phase 2 has covered its columns
    qtrig = [min((QB[q + 1] + GW - 1) // GW, len(groups)) - 1 for q in range(NW)]

    with tile.TileContext(nc) as tc:
        with (
            tc.tile_pool(name="persist", bufs=1) as pp,
            tc.tile_pool(name="pdram", bufs=1, space="DRAM") as pd,
            tc.tile_pool(name="gpool", bufs=8) as gpool,
            tc.tile_pool(name="ohpool", bufs=2) as ohpool,
            tc.tile_pool(name="mpool", bufs=3) as mpool,
            tc.tile_pool(name="grp", bufs=2) as grp,
            tc.tile_pool(name="small", bufs=8) as small,
            tc.tile_pool(name="trp", bufs=4) as trp,
            tc.tile_pool(name="mps", bufs=4, space="PSUM") as mps,
            tc.tile_pool(name="xps", bufs=2, space="PSUM") as xps,
            tc.tile_pool(name="tps", bufs=2, space="PSUM") as tps,
        ):
            # --- persistent SBUF state ---
            bufA = pp.tile([P, NPC_PAD], f32, tag="bufA", name="bufA")
            bufB = pp.tile([P, NPC_PAD], gd, tag="bufB", name="bufB")
            ident = pp.tile([P, P], f32, tag="ident", name="ident")
            make_identity(nc, ident[:])
            eps_t = pp.tile([P, 1], f32, tag="eps_t", name="eps_t")
            nc.gpsimd.memset(eps_t[:], EPS)

            lhs1_t = pp.tile([IN + 1, H], f32, tag="lhs1_t", name="lhs1_t")
            nc.sync.dma_start(lhs1_t[:], lhs1_p[:])
            lhs2_t = pp.tile([IN + 1, H], f32, tag="lhs2_t", name="lhs2_t")
            nc.sync.dma_start(lhs2_t[:], lhs2_p[:])
            W1_t = pp.tile([H, H // 2], f32, tag="W1_t", name="W1_t")
            nc.sync.dma_start(W1_t[:], W1_p[:])
            b1_t = pp.tile([H // 2, 1], f32, tag="b1_t", name="b1_t")
            nc.sync.dma_start(b1_t[:], b1_p[:])
            W2_t = pp.tile([H // 2, OUT], f32, tag="W2_t", name="W2_t")
            nc.sync.dma_start(W2_t[:], W2_p[:])
            b2_t = pp.tile([OUT, 1], f32, tag="b2_t", name="b2_t")
            nc.sync.dma_start(b2_t[:], b2_p[:])

            # --- internal DRAM: per-quarter AG inputs + window tables ---
            ag_q = [
                [pd.tile([QS[q], H], gd, tag=f"ag{l}_{q}", name=f"ag{l}_{q}")
                 for q in range(NW)]
                for l in range(2)
            ]
            tab = [
                [pd.tile([WR[q], H], gd, addr_space="Shared",
                         tag=f"tab{l}_{q}", name=f"tab{l}_{q}")
                 for q in range(NW)]
                for l in range(2)
            ]
            ar_in = [
                pd.tile([P, 2], f32, tag=f"ar_in{l}", name=f"ar_in{l}")
                for l in range(L)
            ]
            ar_out = [
                pd.tile([P, 2], f32, addr_space="Shared",
                        tag=f"ar_out{l}", name=f"ar_out{l}")
                for l in range(L)
            ]

            def transpose_quarter(l, q):
                kt = sum(QT[:q])
                nt_q = QT[q]
                done = 0
                while done < nt_q:
                    nstage = min(4, nt_q - done)
                    stage = trp.tile([P, GW], gd, tag="tr")
                    for k in range(nstage):
                        t = kt + done + k
                        ps = tps.tile([P, P], f32, tag="tps")
                        nc.tensor.transpose(
                            out=ps[:], in_=bufA[:, t * P:(t + 1) * P],
                            identity=ident[:],
                        )
                        nc.vector.tensor_copy(
                            out=stage[:, k * P:(k + 1) * P], in_=ps[:])
                    so = done * P
                    w = nstage * P
                    nc.scalar.dma_start(
                        ag_q[l][q][so:so + w, :].rearrange(
                            "(b p) f -> p b f", p=P),
                        stage[:, :w].rearrange("p (b f) -> p b f", f=P),
                    )
                    done += nstage
                nc.gpsimd.collective_compute(
                    "AllGather",
                    mybir.AluOpType.bypass,
                    ins=[ag_q[l][q].opt()],
                    outs=[tab[l][q].opt()],
                    replica_groups=rg,
                )

            def head_group(s, w):
                ps1 = xps.tile([H // 2, GW], f32, tag="xps")
                nc.tensor.matmul(out=ps1[:, :w], lhsT=W1_t[:],
                                 rhs=bufA[:, s:s + w], start=True, stop=True)
                z1 = grp.tile([H // 2, GW], f32, tag="z1")
                nc.scalar.activation(
                    z1[:, :w], ps1[:, :w],
                    mybir.ActivationFunctionType.Relu, bias=b1_t[:],
                )
                ps2 = tps.tile([OUT, GW], f32, tag="tps")
                nc.tensor.matmul(out=ps2[:, :w], lhsT=W2_t[:],
                                 rhs=z1[:, :w], start=True, stop=True)
                o = trp.tile([OUT, GW], f32, tag="tro")
                nc.vector.tensor_scalar_add(o[:, :w], ps2[:, :w], b2_t[:])
                nc.sync.dma_start(out_p[:, s:s + w], o[:, :w])

            # ---------------- conv layers ----------------
            for l in range(L):
                if l > 0:
                    Wself_t = small.tile([H, H], f32, tag="ws", bufs=2)
                    nc.sync.dma_start(Wself_t[:], Wself_p[l])
                    Wneigh_t = small.tile([H, H], f32, tag="wn", bufs=2)
                    nc.sync.dma_start(Wneigh_t[:], Wneigh_p[l])
                bng_t = small.tile([H, 1], f32, tag="bng", bufs=2)
                nc.sync.dma_start(bng_t[:], bng_p[l])
                bnb_t = small.tile([H, 1], f32, tag="bnb", bufs=2)
                nc.sync.dma_start(bnb_t[:], bnb_p[l])

                s1p = small.tile([P, 32], f32, tag="s1p", bufs=2)
                s2p = small.tile([P, 32], f32, tag="s2p", bufs=2)

                def stats_and_stash(ps, gi, s, w):
                    wr = w if s + w <= NPC else max(0, NPC - s)
                    if wr > 0:
                        nc.vector.reduce_sum(
                            s1p[:, gi:gi + 1], ps[:, :wr],
                            axis=mybir.AxisListType.X,
                        )
                        sq = grp.tile([P, GW], f32, tag="sq")
                        nc.scalar.activation(
                            sq[:, :wr], ps[:, :wr],
                            mybir.ActivationFunctionType.Square,
                            accum_out=s2p[:, gi:gi + 1],
                        )
                    nc.vector.tensor_copy(out=bufB[:, s:s + w], in_=ps[:, :w])

                # ---- phase 1: messages + GEMM + stats ----
                if l == 0:
                    for gi, (s, w) in enumerate(groups):
                        fa = mpool.tile([IN + 1, GW], f32, tag="fa")
                        nc.sync.dma_start(fa[:, :w], featT_p[:, s:s + w])
                        fx = mpool.tile([IN + 1, GW], f32, tag="fx")
                        nc.sync.dma_start(fx[:, :w], hXT_p[:, s:s + w])
                        ps = xps.tile([P, GW], f32, tag="xps")
                        nc.tensor.matmul(out=ps[:, :w], lhsT=lhs1_t[:],
                                         rhs=fa[:, :w], start=True, stop=False)
                        nc.tensor.matmul(out=ps[:, :w], lhsT=lhs2_t[:],
                                         rhs=fx[:, :w], start=False, stop=True)
                        stats_and_stash(ps, gi, s, w)
                else:
                    for v, (g0, ngr) in enumerate(WAVES):
                        ngtiles = 8 if ngr == 2 else 2
                        wcols = ngtiles * CAPS // 16
                        nidx = ngtiles * CAPS
                        it = mpool.tile([P, 4 * WCMAX], i16, tag="idx")
                        nc.sync.dma_start(it[:, :4 * wcols],
                                          idx_p[v][:, :4 * wcols])
                        oh_t = ohpool.tile([P, CPW * P], gd, tag="oh")
                        ncols = ngtiles * NW * C_tw * P
                        oh_eng = nc.sync if v % 2 == 0 else nc.scalar
                        oh_eng.dma_start(oh_t[:, :ncols], oh_p[v][:, :ncols])
                        # one gather per (window, group): 4*CAPS idxs each
                        # stays under the 128-per-engine SWDGE ring window
                        gcols = 4 * CAPS // 16
                        Gs = {}
                        for gw in range(ngr):
                            for wi in range(NW):
                                gt_n = min(4, ngtiles - gw * 4)
                                ni = gt_n * CAPS
                                gw_t = gpool.tile([P, 4 * C_tw * H], gd,
                                                  tag="g")
                                cb = wi * wcols + gw * gcols
                                nc.gpsimd.dma_gather(
                                    out_ap=gw_t[:, :ni // P * H].rearrange(
                                        "p (c e) -> p c e", e=H),
                                    in_ap=tab[l - 1][wi][:],
                                    idxs_ap=it[:, cb:cb + ni // 16],
                                    num_idxs=ni,
                                    num_idxs_reg=ni,
                                    elem_size=H,
                                    queue_num=wi,
                                )
                                Gs[(wi, gw)] = gw_t
                        for gw in range(ngr):
                            gi = g0 + gw
                            s, w = groups[gi]
                            ntg = (w + P - 1) // P
                            msg_ps = mps.tile([P, GW], f32, tag="mps")
                            nmm = NW * C_tw
                            for t in range(ntg):
                                i_mm = 0
                                for wi in range(NW):
                                    for cc2 in range(C_tw):
                                        gt = gw * 4 + t
                                        ci = t * C_tw + cc2
                                        m = (gt * NW + wi) * C_tw + cc2
                                        nc.tensor.matmul(
                                            out=msg_ps[:, t * P:(t + 1) * P],
                                            lhsT=Gs[(wi, gw)][:,
                                                             ci * H:(ci + 1) * H],
                                            rhs=oh_t[:, m * P:(m + 1) * P],
                                            start=(i_mm == 0),
                                            stop=(i_mm == nmm - 1),
                                        )
                                        i_mm += 1
                            msg_g = grp.tile([P, GW], f32, tag="msg")
                            nc.scalar.copy(out=msg_g[:, :w], in_=msg_ps[:, :w])
                            ps = xps.tile([P, GW], f32, tag="xps")
                            nc.tensor.matmul(out=ps[:, :w], lhsT=Wself_t[:],
                                             rhs=bufA[:, s:s + w],
                                             start=True, stop=False)
                            nc.tensor.matmul(out=ps[:, :w], lhsT=Wneigh_t[:],
                                             rhs=msg_g[:, :w],
                                             start=False, stop=True)
                            stats_and_stash(ps, gi, s, w)

                # ---- BN stats all-reduce ----
                st = small.tile([P, 2], f32, tag="st", bufs=2)
                nc.vector.reduce_sum(st[:, 0:1], s1p[:, :len(groups)],
                                     axis=mybir.AxisListType.X)
                nc.vector.reduce_sum(st[:, 1:2], s2p[:, :len(groups)],
                                     axis=mybir.AxisListType.X)
                nc.sync.dma_start(ar_in[l][:], st[:])
                nc.gpsimd.collective_compute(
                    "AllReduce",
                    mybir.AluOpType.add,
                    ins=[ar_in[l].opt()],
                    outs=[ar_out[l].opt()],
                    replica_groups=rg,
                )
                sg = small.tile([P, 2], f32, tag="sg", bufs=2)
                nc.sync.dma_start(sg[:], ar_out[l][:])

                mu = small.tile([P, 1], f32, tag="mu", bufs=2)
                nc.vector.tensor_scalar_mul(mu[:], sg[:, 0:1], 1.0 / N)
                ex2 = small.tile([P, 1], f32, tag="ex2", bufs=2)
                nc.vector.tensor_scalar_mul(ex2[:], sg[:, 1:2], 1.0 / N)
                var = small.tile([P, 1], f32, tag="var", bufs=2)
                nc.vector.tensor_tensor(out=var[:], in0=mu[:], in1=mu[:],
                                        op=mybir.AluOpType.mult)
                nc.vector.tensor_tensor(out=var[:], in0=ex2[:], in1=var[:],
                                        op=mybir.AluOpType.subtract)
                sd = small.tile([P, 1], f32, tag="sd", bufs=2)
                nc.scalar.activation(sd[:], var[:],
                                     mybir.ActivationFunctionType.Sqrt,
                                     bias=eps_t[:])
                rstd = small.tile([P, 1], f32, tag="rstd", bufs=2)
                nc.vector.reciprocal(rstd[:], sd[:])
                a_t = small.tile([P, 1], f32, tag="a_t", bufs=2)
                nc.vector.tensor_tensor(out=a_t[:], in0=bng_t[:], in1=rstd[:],
                                        op=mybir.AluOpType.mult)
                b_t = small.tile([P, 1], f32, tag="b_t", bufs=2)
                nc.vector.tensor_tensor(out=b_t[:], in0=mu[:], in1=a_t[:],
                                        op=mybir.AluOpType.mult)
                nc.vector.tensor_tensor(out=b_t[:], in0=bnb_t[:], in1=b_t[:],
                                        op=mybir.AluOpType.subtract)

                # ---- phase 2: fused BN+ReLU (+ residual), interleaved with
                # per-quarter transposes + chunked AllGather ----
                nextq = 0
                for gi, (s, w) in enumerate(groups):
                    if l == 0:
                        nc.scalar.activation(
                            bufA[:, s:s + w], bufB[:, s:s + w],
                            mybir.ActivationFunctionType.Relu,
                            bias=b_t[:], scale=a_t[:],
                        )
                    else:
                        y2 = grp.tile([P, GW], f32, tag="y2")
                        nc.scalar.activation(
                            y2[:, :w], bufB[:, s:s + w],
                            mybir.ActivationFunctionType.Relu,
                            bias=b_t[:], scale=a_t[:],
                        )
                        nc.vector.tensor_tensor(
                            out=bufA[:, s:s + w], in0=y2[:, :w],
                            in1=bufA[:, s:s + w], op=mybir.AluOpType.add,
                        )
                    if gi == len(groups) - 1:
                        nc.gpsimd.memset(bufA[:, NPC:], 0.0)
                    if l == 2:
                        head_group(s, w)
                    elif nextq < NW and gi == qtrig[nextq]:
                        if gi == len(groups) - 1:
                            pass  # memset already issued above
                        transpose_quarter(l, nextq)
                        nextq += 1

    nc.compile()
    return nc


def kernel(**inputs):
    from concourse.bass_utils import run_bass_kernel_spmd

    idx, oh, featT, hXT, lhs1, lhs2, C = _host_prep(**inputs)

    key = ("prog", C)
    if key not in _cache:
        _cache[key] = _build_program(C)
    nc = _cache[key]

    f32 = np.float32
    Wself = np.ascontiguousarray(np.asarray(inputs["Wself"], f32))
    Wneigh = np.ascontiguousarray(np.asarray(inputs["Wneigh"], f32))
    bng = np.asarray(inputs["bn_gamma"], f32).reshape(L, H, 1)
    bnb = np.asarray(inputs["bn_beta"], f32).reshape(L, H, 1)
    W1 = np.ascontiguousarray(np.asarray(inputs["W1"], f32))
    b1 = np.asarray(inputs["b1"], f32).reshape(H // 2, 1)
    W2 = np.ascontiguousarray(np.asarray(inputs["W2"], f32))
    b2 = np.asarray(inputs["b2"], f32).reshape(OUT, 1)

    in_maps = []
    for c in range(NCORES):
        in_maps.append({
            "featT": np.ascontiguousarray(featT[c]),
            "hXT": np.ascontiguousarray(hXT[c]),
            "lhs1": np.ascontiguousarray(lhs1),
            "lhs2": np.ascontiguousarray(lhs2),
            "idx": np.ascontiguousarray(idx[c]),
            "oh": np.ascontiguousarray(oh[c]),
            "Wself": Wself, "Wneigh": Wneigh,
            "bng": bng, "bnb": bnb,
            "W1": W1, "b1": b1, "W2": W2, "b2": b2,
        })

    global _last_in_maps
    _last_in_maps = in_maps

    res = run_bass_kernel_spmd(nc, in_maps, list(range(NCORES))).results
    out = np.concatenate(
        [res[c]["out"][:, :NPC].T for c in range(NCORES)], axis=0
    )
    return out.astype(np.float32)


if __name__ == "__main__":
    pass
